# revision 23
# baseline (speedup 1.0000x reference)
"""Trainium2 Bass kernel for nn_LocalAttention (5x5 local window attention).

Contract: kernel(**inputs) takes the FULL inputs from setup_inputs() and
returns the FULL output.  Internally shards across 8 NeuronCores as
(batch b in 0..3) x (head-group hg in 0..1, 4 heads each).  Each core
computes a partial output projection; the host sums the two partials per
batch and adds b_out.

Structure (v2 — software-pipelined):
  B: qkv projection of 512-px blocks into persistent qt2/kt2 (d-major,
     fp16, [128, 2g, n]) and vsb (pixel-major fp16 [128, chunk, 4*(64+1)]
     with ones columns for the softmax denominator, written contiguously
     via zero-padded v-weights + indicator add).  All 8 x-block DMAs are
     issued up front.  q-pair casts on ACT, k-pair on DVE.
  C: skewed pipeline (stage_a(i+1) emitted before stage_b(i)).
     stage_a: banded transposed dots into 3 PSUM tiles (edge chunks c0/c3
     fused into one tile), exp on ACT, window-mask multiply (edge mask on
     GpSimd, middles on DVE) -> em.  stage_b: 3-chunk accumulating AV
     matmul per (head, pixel-half) with ones column giving the softmax
     denominator, normalize, PE transpose, out-projection (half-width
     PSUM tiles), fp16 DMA out.  Column-wrapped window positions are
     masked out and re-added to the denominator via n_pad.
"""

import numpy as np

B, HMAP, WMAP = 4, 64, 64
N = HMAP * WMAP          # 4096
DIM = 512
HEADS, HEAD_DIM = 8, 64
INNER = HEADS * HEAD_DIM  # 512
SCALE = HEAD_DIM ** -0.5
NB = N + 256             # padded k/v buffer pixels (2 zero rows each side)
NCHUNK = NB // 128       # 34
N_CORES = 8

_cache = {}


def _make_masks():
    """Window/wrap masks for the 4 chunks of a 256-px batch, plus n_pad.

    mask[c, j', p'] = 1 iff o = 128*c + j' - p' - 128 decomposes as
    64*di + dj with |di|,|dj| <= 2 and column p'%64 + dj stays in-image.
    n_pad[p] = number of column-invalid window positions for column p%64.
    """
    o = (128 * np.arange(4)[:, None, None] + np.arange(128)[None, :, None]
         - np.arange(256)[None, None, :] - 128)           # [4,128,256]
    di = np.round(o / 64.0).astype(np.int64)
    dj = o - 64 * di
    col = (np.arange(256) % 64)[None, None, :]
    ok = (np.abs(di) <= 2) & (np.abs(dj) <= 2) & (col + dj >= 0) & (col + dj < 64)
    masks = ok.astype(np.float16)
    colv = np.arange(64)
    npad_col = np.zeros(64, dtype=np.float32)
    for djv in range(-2, 3):
        npad_col += 5.0 * ((colv + djv < 0) | (colv + djv >= 64))
    n_pad = np.tile(npad_col, 2).reshape(128, 1).astype(np.float32)
    # fused edge mask: c0 for p'<128, c3 for p'>=128 (those are the only
    # halves the edge chunks serve)
    m03 = np.concatenate([masks[0][:, 0:128], masks[3][:, 128:256]], axis=1)
    masks3 = np.stack([m03, masks[1], masks[2]], axis=1)  # [128, 3, 256]
    return np.ascontiguousarray(masks3), n_pad


def _build_nc():
    import concourse.bass as bass
    import concourse.tile as tile
    from concourse import mybir

    f32 = mybir.dt.float32
    f16 = mybir.dt.float16
    Exp = mybir.ActivationFunctionType.Exp

    from concourse import bacc
    nc = bacc.Bacc(None, target_bir_lowering=False)
    # xt/wqkvt/masks come pre-blocked from the host so every DMA descriptor
    # is a contiguous >=2KB per-partition run.
    xt_d = nc.dram_tensor("xt", [8, 128, 4, 512], f16, kind="ExternalInput")
    wqkvt_d = nc.dram_tensor("wqkvt", [128, 4, 772], f16, kind="ExternalInput")
    woutt_d = nc.dram_tensor("woutt", [256, DIM], f16, kind="ExternalInput")
    masks_d = nc.dram_tensor("masks", [128, 3, 256], f16, kind="ExternalInput")
    npad_d = nc.dram_tensor("npad", [128, 1], f32, kind="ExternalInput")
    ident_d = nc.dram_tensor("ident", [128, 128], f16, kind="ExternalInput")
    out_d = nc.dram_tensor("out", [N, DIM], f16, kind="ExternalOutput")

    with tile.TileContext(nc) as tc:
        from contextlib import ExitStack
        with ExitStack() as ctx:
            consts = ctx.enter_context(tc.tile_pool(name="consts", bufs=1))

            # all x blocks live in SBUF; DMAs for every block are issued up
            # front so phase B is never DMA-gated
            xsb = consts.tile([128, 8, 4, 512], f16)
            wqkvt = consts.tile([128, 4, 772], f16)
            woutt = consts.tile([128, 2, DIM], f16)
            masks = consts.tile([128, 3, 256], f16)
            masksf = consts.tile([128, 3, 4, 256], f16)
            npad = consts.tile([128, 1], f32)
            ident = consts.tile([128, 128], f16)
            ind260 = consts.tile([128, 260], f16)

            # persistent activations: [128 part = 2heads x 64d, g, n]
            qt2 = consts.tile([128, 2, N], f16)
            kt2 = consts.tile([128, 2, NB], f16)
            # v buffer: [p, chunk, 4 heads x (64 + ones col)]
            vsb = consts.tile([128, NCHUNK, 260], f16)

            # startup-critical DMA order: kc0 weight slice + kc0 x chunk
            # first (the first matmul needs only those), then the rest
            nc.sync.dma_start(out=wqkvt[:, 0], in_=wqkvt_d[:, 0])
            nc.sync.dma_start(out=xsb[:, 0, 0], in_=xt_d[0][:, 0])
            for kc in range(1, 4):
                nc.sync.dma_start(out=wqkvt[:, kc], in_=wqkvt_d[:, kc])
                nc.sync.dma_start(out=xsb[:, 0, kc], in_=xt_d[0][:, kc])
            for blk in range(1, 8):
                nc.sync.dma_start(out=xsb[:, blk], in_=xt_d[blk])
            nc.sync.dma_start(
                out=woutt,
                in_=woutt_d.rearrange("(c p) m -> p c m", p=128))
            nc.sync.dma_start(out=masks, in_=masks_d[:, :, :])
            nc.sync.dma_start(out=npad, in_=npad_d[:, :])
            nc.sync.dma_start(out=ident, in_=ident_d[:, :])

            # constant prep (contiguous per-partition runs, DVE)
            for g in range(2):
                nc.vector.memset(kt2[:, g, 0:128], 0.0)
                nc.vector.memset(kt2[:, g, NB - 128:NB], 0.0)
            nc.vector.memset(vsb[:, 0, :], 0.0)
            nc.vector.memset(vsb[:, NCHUNK - 1, :], 0.0)
            # pad chunks still need the softmax-denominator ones columns
            # (reference counts zero-padded neighbors as exp(0)=1)
            for ci in (0, NCHUNK - 1):
                nc.vector.memset(
                    vsb[:, ci].rearrange("p (h e) -> p h e", h=4)[:, :, 64:65],
                    1.0)
            nc.vector.memset(ind260, 0.0)
            ind_ap = ind260.rearrange("p (h e) -> p h e", h=4)[:, :, 64:65]
            nc.vector.memset(ind_ap, 1.0)
            # per-chunk masks replicated across the 4 head slots (dense
            # elementwise operand; broadcast APs run slower on DVE/GpSimd)
            for c3 in range(3):
                nc.gpsimd.tensor_copy(
                    masksf[:, c3],
                    masks[:, c3].unsqueeze(1).to_broadcast([128, 4, 256]))

            # ---------------- Phase B: q/k projections ----------------
            # (v projection is folded into phase C where the PE has slack)
            with ExitStack() as bctx:
                psqk = bctx.enter_context(
                    tc.tile_pool(name="psum_qk", bufs=2, space="PSUM"))
                for blk in range(8):
                    s0 = blk * 512
                    xtile = xsb[:, blk]
                    # q pair then k pair; cast q on ACT, k on DVE
                    for pair in range(2):  # 0: q (m 0,1), 1: k (m 2,3)
                        ps = psqk.tile([128, 2, 512], f32, tag="psqk")
                        for m2 in range(2):
                            m = pair * 2 + m2
                            for kc in range(4):
                                nc.tensor.matmul(
                                    ps[:, m2],
                                    wqkvt[:, kc, m * 128:(m + 1) * 128],
                                    xtile[:, kc, :],
                                    start=(kc == 0), stop=(kc == 3))
                        if pair == 0:
                            nc.scalar.copy(qt2[:, :, s0:s0 + 512], ps)
                        else:
                            nc.vector.tensor_copy(
                                kt2[:, :, 128 + s0:128 + s0 + 512], ps)

            # ---------------- Phase C: attention + projection ----------------
            with ExitStack() as cctx:
                pwp = cctx.enter_context(
                    tc.tile_pool(name="psum_w", bufs=2, space="PSUM"))
                pso = cctx.enter_context(
                    tc.tile_pool(name="psum_o", bufs=1, space="PSUM"))
                pspj = cctx.enter_context(
                    tc.tile_pool(name="psum_pj", bufs=1, space="PSUM"))
                psvp = cctx.enter_context(
                    tc.tile_pool(name="psum_v", bufs=1, space="PSUM"))

                def v_chunk(ci):
                    """JIT v projection of 128-px chunk ci into vsb"""
                    blk, sub = (ci - 1) // 4, (ci - 1) % 4
                    psv = psvp.tile([128, 260], f32, tag="psv")
                    for kc in range(4):
                        nc.tensor.matmul(
                            psv,
                            xsb[:, blk, kc, sub * 128:(sub + 1) * 128],
                            wqkvt[:, kc, 512:772],
                            start=(kc == 0), stop=(kc == 3))
                    # contiguous write; ones columns come from the
                    # indicator add over the zero-padded weight cols
                    nc.vector.tensor_add(vsb[:, ci, :], psv, ind260)
                epool = cctx.enter_context(tc.tile_pool(name="em", bufs=2))
                erpool = cctx.enter_context(tc.tile_pool(name="er", bufs=3))
                opool = cctx.enter_context(tc.tile_pool(name="oc", bufs=2))
                dpool = cctx.enter_context(tc.tile_pool(name="den", bufs=2))

                def dots_mms(pw, g, hs, ksl, qsl, sub):
                    nc.tensor.matmul(
                        pw[:, hs, g, sub],
                        kt2[64 * hs:64 * hs + 64, g, ksl],
                        qt2[64 * hs:64 * hs + 64, g, qsl],
                        start=True, stop=True)

                def stage_a(si):
                    """dots + exp + mask for batch si -> em tile;
                    also projects the two v chunks stage_b(si) will need
                    beyond what earlier stage_a calls produced."""
                    s = si * 256
                    for ci in (2 * si + 1, 2 * si + 2):
                        if 1 <= ci <= 32:
                            v_chunk(ci)
                    em = epool.tile([128, 3, 4, 256], f16, tag="em",
                                    name=f"em{si}")
                    # chunk 0 = fused edges (c0 at p 0:128, c3 at p 128:256),
                    # chunks 1/2 = middles.  exp+mask are emitted right after
                    # each chunk's matmuls so the ring-2 pw pool can recycle.
                    # er12 collects both middle exps so their mask multiply
                    # runs as one 2x-mode DVE op; edge mask goes to GpSimd.
                    er12 = erpool.tile([128, 2, 4, 256], f16, tag="er12")
                    for idx in range(3):
                        pt = pwp.tile([128, 2, 2, 256], f32, tag="pw")
                        for g in range(2):
                            for hs in range(2):
                                if idx == 0:
                                    dots_mms(pt, g, hs,
                                             slice(s, s + 128),
                                             slice(s, s + 128),
                                             slice(0, 128))
                                    dots_mms(pt, g, hs,
                                             slice(s + 384, s + 512),
                                             slice(s + 128, s + 256),
                                             slice(128, 256))
                                else:
                                    dots_mms(pt, g, hs,
                                             slice(s + 128 * idx,
                                                   s + 128 * idx + 128),
                                             slice(s, s + 256),
                                             slice(0, 256))
                        if idx == 0:
                            er = erpool.tile([128, 4, 256], f16, tag="er")
                        else:
                            er = er12[:, idx - 1]
                        nc.scalar.activation(
                            out=er,
                            in_=pt.rearrange("p a b f -> p (a b) f"),
                            func=Exp, scale=SCALE)
                        if idx == 0:
                            nc.gpsimd.tensor_mul(em[:, 0], er, masksf[:, 0])
                    nc.vector.tensor_mul(em[:, 1:3], er12, masksf[:, 1:3])
                    return em

                def stage_b(si, em):
                    """AV + normalize + transpose + projection + out"""
                    s = si * 256
                    # ph stride padded to a full 2KB bank: a matmul PSUM
                    # target must not cross a bank boundary
                    po_raw = pso.tile([128, 2, 512], f32, tag="po")
                    po = po_raw[:, :, 0:260].rearrange(
                        "p a (g e) -> p a g e", e=65)
                    for gh in range(4):
                        slot = 2 * (gh % 2) + gh // 2
                        for ph in range(2):
                            # chunk index within em (0 = fused edge)
                            cs = ((0, 1, 2) if ph == 0 else (1, 2, 0))
                            vcs = ((0, 1, 2) if ph == 0 else (1, 2, 3))
                            for i in range(3):
                                nc.tensor.matmul(
                                    po[:, ph, gh, :],
                                    em[:, cs[i], slot,
                                       ph * 128:(ph + 1) * 128],
                                    vsb[:, 2 * si + vcs[i],
                                        65 * gh:65 * gh + 65],
                                    start=(i == 0), stop=(i == 2))
                    den = dpool.tile([128, 2, 4], f32, tag="den")
                    nc.vector.tensor_add(
                        den.unsqueeze(3),
                        po[:, :, :, 64:65],
                        npad.unsqueeze(2).unsqueeze(3).to_broadcast(
                            [128, 2, 4, 1]))
                    rec = dpool.tile([128, 2, 4], f32, tag="rec")
                    nc.vector.reciprocal(rec, den)

                    ob = opool.tile([128, 2, DIM], f16, tag="ob")
                    for ph in range(2):
                        opix = opool.tile([128, 256], f16, tag="opix")
                        nc.vector.tensor_mul(
                            opix.rearrange("p (g e) -> p g e", g=4),
                            po[:, ph, :, 0:64],
                            rec[:, ph, :].unsqueeze(2).to_broadcast(
                                [128, 4, 64]))
                        otb = opool.tile([128, 2, 128], f16, tag="otb")
                        # transposes land in po_raw's pad hole (disjoint from
                        # the attention columns) to save a PSUM bank
                        pt = po_raw[:, ph, 320:448].bitcast(f16).rearrange(
                            "p (i f) -> p i f", i=2)
                        for i in range(2):
                            nc.tensor.transpose(
                                pt[:, i], opix[:, i * 128:(i + 1) * 128],
                                ident)
                        nc.vector.tensor_copy(otb, pt)
                        pj = pspj.tile([128, DIM], f32, tag="pj")
                        for i in range(2):
                            nc.tensor.matmul(
                                pj, otb[:, i], woutt[:, i],
                                start=(i == 0), stop=(i == 1))
                        # casts alternate ACT/DVE for balance
                        if ph == 0:
                            nc.scalar.copy(ob[:, 0], pj)
                        else:
                            nc.vector.tensor_copy(ob[:, 1], pj)
                    # one DMA for both 128-px halves (fewer DMA instrs +
                    # semaphore waits)
                    nc.sync.dma_start(
                        out=out_d[s:s + 256, :].rearrange(
                            "(h p) m -> p h m", p=128),
                        in_=ob)

                # skewed software pipeline: dots/exp/mask of batch i+1 are
                # emitted before AV/proj of batch i so the PE never waits
                # on the ACT exp chain
                em_prev = stage_a(0)
                for si in range(16):
                    em_next = stage_a(si + 1) if si < 15 else None
                    stage_b(si, em_prev)
                    em_prev = em_next

    nc.finalize()
    return nc


def _prepare_core_inputs(x, w_qkv, w_out, b_out):
    masks3, n_pad = _make_masks()
    masks_p = np.ascontiguousarray(masks3)        # [128, 3, 256]
    ident = np.eye(128, dtype=np.float16)
    per_core = []
    for ci in range(N_CORES):
        b, hg = ci // 2, ci % 2
        q_rows = w_qkv[256 * hg:256 * hg + 256]
        k_rows = w_qkv[INNER + 256 * hg:INNER + 256 * hg + 256]
        v_rows = w_qkv[2 * INNER + 256 * hg:2 * INNER + 256 * hg + 256]
        # v rows padded with zero columns at the ones positions
        v_pad = np.zeros((260, 512), dtype=w_qkv.dtype)
        for h in range(4):
            v_pad[65 * h:65 * h + 64] = v_rows[64 * h:64 * h + 64]
        w_slice = np.concatenate([q_rows, k_rows, v_pad], axis=0)  # [772, 512]
        xt = x[b].T.astype(np.float16)                       # [512, 4096]
        # blocked: [blk, p, c, n-slice] so each per-partition DMA run is 4KB
        xtb = np.ascontiguousarray(
            xt.reshape(4, 128, 8, 512).transpose(2, 1, 0, 3))
        wq = np.ascontiguousarray(
            w_slice.T.astype(np.float16).reshape(4, 128, 772).transpose(1, 0, 2))
        per_core.append({
            "xt": xtb,
            "wqkvt": wq,
            "woutt": np.ascontiguousarray(
                w_out[:, 256 * hg:256 * hg + 256].T).astype(np.float16),
            "masks": masks_p,
            "npad": n_pad,
            "ident": ident,
        })
    return per_core


def kernel(x, w_qkv, w_out, b_out, h, w):
    assert int(h) == HMAP and int(w) == WMAP
    x = np.asarray(x, dtype=np.float32)
    w_qkv = np.asarray(w_qkv, dtype=np.float32)
    w_out = np.asarray(w_out, dtype=np.float32)
    b_out = np.asarray(b_out, dtype=np.float32)

    if "nc" not in _cache:
        _cache["nc"] = _build_nc()
    nc = _cache["nc"]

    from concourse.bass_utils import run_bass_kernel_spmd
    in_maps = _prepare_core_inputs(x, w_qkv, w_out, b_out)
    res = run_bass_kernel_spmd(nc, in_maps, core_ids=list(range(N_CORES)))
    out = np.zeros((B, N, DIM), dtype=np.float32)
    for b in range(B):
        out[b] = (res.results[2 * b]["out"].astype(np.float32)
                  + res.results[2 * b + 1]["out"].astype(np.float32)
                  + b_out[None, :])
    return out


# revision 24
# speedup vs baseline: 1.0091x; 1.0091x over previous
"""Trainium2 Bass kernel for nn_LocalAttention (5x5 local window attention).

Contract: kernel(**inputs) takes the FULL inputs from setup_inputs() and
returns the FULL output.  Internally shards across 8 NeuronCores as
(batch b in 0..3) x (head-group hg in 0..1, 4 heads each).  Each core
computes a partial output projection; the host sums the two partials per
batch and adds b_out.

Structure (v2 — software-pipelined):
  B: qkv projection of 512-px blocks into persistent qt2/kt2 (d-major,
     fp16, [128, 2g, n]) and vsb (pixel-major fp16 [128, chunk, 4*(64+1)]
     with ones columns for the softmax denominator, written contiguously
     via zero-padded v-weights + indicator add).  All 8 x-block DMAs are
     issued up front.  q-pair casts on ACT, k-pair on DVE.
  C: skewed pipeline (stage_a(i+1) emitted before stage_b(i)).
     stage_a: banded transposed dots into 3 PSUM tiles (edge chunks c0/c3
     fused into one tile), exp on ACT, window-mask multiply (edge mask on
     GpSimd, middles on DVE) -> em.  stage_b: 3-chunk accumulating AV
     matmul per (head, pixel-half) with ones column giving the softmax
     denominator, normalize, PE transpose, out-projection (half-width
     PSUM tiles), fp16 DMA out.  Column-wrapped window positions are
     masked out and re-added to the denominator via n_pad.
"""

import numpy as np

B, HMAP, WMAP = 4, 64, 64
N = HMAP * WMAP          # 4096
DIM = 512
HEADS, HEAD_DIM = 8, 64
INNER = HEADS * HEAD_DIM  # 512
SCALE = HEAD_DIM ** -0.5
NB = N + 256             # padded k/v buffer pixels (2 zero rows each side)
NCHUNK = NB // 128       # 34
N_CORES = 8

_cache = {}


def _make_masks():
    """Window/wrap masks for the 4 chunks of a 256-px batch, plus n_pad.

    mask[c, j', p'] = 1 iff o = 128*c + j' - p' - 128 decomposes as
    64*di + dj with |di|,|dj| <= 2 and column p'%64 + dj stays in-image.
    n_pad[p] = number of column-invalid window positions for column p%64.
    """
    o = (128 * np.arange(4)[:, None, None] + np.arange(128)[None, :, None]
         - np.arange(256)[None, None, :] - 128)           # [4,128,256]
    di = np.round(o / 64.0).astype(np.int64)
    dj = o - 64 * di
    col = (np.arange(256) % 64)[None, None, :]
    ok = (np.abs(di) <= 2) & (np.abs(dj) <= 2) & (col + dj >= 0) & (col + dj < 64)
    masks = ok.astype(np.float16)
    colv = np.arange(64)
    npad_col = np.zeros(64, dtype=np.float32)
    for djv in range(-2, 3):
        npad_col += 5.0 * ((colv + djv < 0) | (colv + djv >= 64))
    n_pad = np.tile(npad_col, 2).reshape(128, 1).astype(np.float32)
    # fused edge mask: c0 for p'<128, c3 for p'>=128 (those are the only
    # halves the edge chunks serve)
    m03 = np.concatenate([masks[0][:, 0:128], masks[3][:, 128:256]], axis=1)
    masks3 = np.stack([m03, masks[1], masks[2]], axis=1)  # [128, 3, 256]
    return np.ascontiguousarray(masks3), n_pad


def _build_nc():
    import concourse.bass as bass
    import concourse.tile as tile
    from concourse import mybir

    f32 = mybir.dt.float32
    f16 = mybir.dt.float16
    Exp = mybir.ActivationFunctionType.Exp

    from concourse import bacc
    nc = bacc.Bacc(None, target_bir_lowering=False)
    # xt/wqkvt/masks come pre-blocked from the host so every DMA descriptor
    # is a contiguous >=2KB per-partition run.
    xt_d = nc.dram_tensor("xt", [8, 128, 4, 512], f16, kind="ExternalInput")
    wqkvt_d = nc.dram_tensor("wqkvt", [128, 4, 768], f16, kind="ExternalInput")
    woutt_d = nc.dram_tensor("woutt", [256, DIM], f16, kind="ExternalInput")
    masks_d = nc.dram_tensor("masks", [128, 3, 256], f16, kind="ExternalInput")
    npad_d = nc.dram_tensor("npad", [128, 1], f32, kind="ExternalInput")
    ident_d = nc.dram_tensor("ident", [128, 128], f16, kind="ExternalInput")
    out_d = nc.dram_tensor("out", [N, DIM], f16, kind="ExternalOutput")

    with tile.TileContext(nc) as tc:
        from contextlib import ExitStack
        with ExitStack() as ctx:
            consts = ctx.enter_context(tc.tile_pool(name="consts", bufs=1))

            # all x blocks live in SBUF; DMAs for every block are issued up
            # front so phase B is never DMA-gated
            xsb = consts.tile([128, 8, 4, 512], f16)
            wqkvt = consts.tile([128, 4, 768], f16)
            woutt = consts.tile([128, 2, DIM], f16)
            masks = consts.tile([128, 3, 256], f16)
            masksf = consts.tile([128, 3, 4, 256], f16)
            npad = consts.tile([128, 1], f32)
            ident = consts.tile([128, 128], f16)

            # persistent activations: [128 part = 2heads x 64d, g, n]
            qt2 = consts.tile([128, 2, N], f16)
            kt2 = consts.tile([128, 2, NB], f16)
            # v buffer: [p, chunk, 4 heads x (64 + ones col)]
            vsb = consts.tile([128, NCHUNK, 260], f16)

            # startup-critical DMA order: kc0 weight slice + kc0 x chunk
            # first (the first matmul needs only those), then the rest
            nc.sync.dma_start(out=wqkvt[:, 0], in_=wqkvt_d[:, 0])
            nc.sync.dma_start(out=xsb[:, 0, 0], in_=xt_d[0][:, 0])
            for kc in range(1, 4):
                nc.sync.dma_start(out=wqkvt[:, kc], in_=wqkvt_d[:, kc])
                nc.sync.dma_start(out=xsb[:, 0, kc], in_=xt_d[0][:, kc])
            for blk in range(1, 8):
                nc.sync.dma_start(out=xsb[:, blk], in_=xt_d[blk])
            nc.sync.dma_start(
                out=woutt,
                in_=woutt_d.rearrange("(c p) m -> p c m", p=128))
            nc.sync.dma_start(out=masks, in_=masks_d[:, :, :])
            nc.sync.dma_start(out=npad, in_=npad_d[:, :])
            nc.sync.dma_start(out=ident, in_=ident_d[:, :])

            # constant prep (contiguous per-partition runs, DVE)
            for g in range(2):
                nc.vector.memset(kt2[:, g, 0:128], 0.0)
                nc.vector.memset(kt2[:, g, NB - 128:NB], 0.0)
            nc.vector.memset(vsb[:, 0, :], 0.0)
            nc.vector.memset(vsb[:, NCHUNK - 1, :], 0.0)
            # softmax-denominator ones columns for every chunk (zero-padded
            # neighbors count as exp(0)=1 in the reference)
            nc.vector.memset(
                vsb.rearrange("p c (h e) -> p c h e", h=4)[:, :, :, 64:65],
                1.0)
            # per-chunk masks replicated across the 4 head slots (dense
            # elementwise operand; broadcast APs run slower on DVE/GpSimd)
            for c3 in range(3):
                nc.gpsimd.tensor_copy(
                    masksf[:, c3],
                    masks[:, c3].unsqueeze(1).to_broadcast([128, 4, 256]))

            # ---------------- Phase B: q/k projections ----------------
            # (v projection is folded into phase C where the PE has slack)
            with ExitStack() as bctx:
                psqk = bctx.enter_context(
                    tc.tile_pool(name="psum_qk", bufs=2, space="PSUM"))
                for blk in range(8):
                    s0 = blk * 512
                    xtile = xsb[:, blk]
                    # q pair then k pair; cast q on ACT, k on DVE
                    for pair in range(2):  # 0: q (m 0,1), 1: k (m 2,3)
                        ps = psqk.tile([128, 2, 512], f32, tag="psqk")
                        for m2 in range(2):
                            m = pair * 2 + m2
                            for kc in range(4):
                                nc.tensor.matmul(
                                    ps[:, m2],
                                    wqkvt[:, kc, m * 128:(m + 1) * 128],
                                    xtile[:, kc, :],
                                    start=(kc == 0), stop=(kc == 3))
                        if pair == 0:
                            nc.scalar.copy(qt2[:, :, s0:s0 + 512], ps)
                        else:
                            nc.vector.tensor_copy(
                                kt2[:, :, 128 + s0:128 + s0 + 512], ps)

            # ---------------- Phase C: attention + projection ----------------
            with ExitStack() as cctx:
                pwp = cctx.enter_context(
                    tc.tile_pool(name="psum_w", bufs=2, space="PSUM"))
                pso = cctx.enter_context(
                    tc.tile_pool(name="psum_o", bufs=1, space="PSUM"))
                pspj = cctx.enter_context(
                    tc.tile_pool(name="psum_pj", bufs=1, space="PSUM"))
                psvp = cctx.enter_context(
                    tc.tile_pool(name="psum_v", bufs=1, space="PSUM"))

                def v_pair(ci):
                    """JIT v projection of 128-px chunks ci, ci+1 into vsb.
                    Both chunks share one single-bank PSUM tile so the next
                    batch's v matmuls only wait on a cast a full period old."""
                    psv = psvp.tile([128, 2, 256], f32, tag="psv")
                    for idx in range(2):
                        blk, sub = (ci + idx - 1) // 4, (ci + idx - 1) % 4
                        for kc in range(4):
                            nc.tensor.matmul(
                                psv[:, idx],
                                xsb[:, blk, kc, sub * 128:(sub + 1) * 128],
                                wqkvt[:, kc, 512:768],
                                start=(kc == 0), stop=(kc == 3))
                    nc.vector.tensor_copy(
                        vsb[:, ci:ci + 2].rearrange(
                            "p c (h e) -> p c h e", h=4)[:, :, :, 0:64],
                        psv.rearrange("p c (h e) -> p c h e", h=4))
                epool = cctx.enter_context(tc.tile_pool(name="em", bufs=2))
                erpool = cctx.enter_context(tc.tile_pool(name="er", bufs=3))
                opool = cctx.enter_context(tc.tile_pool(name="oc", bufs=2))
                dpool = cctx.enter_context(tc.tile_pool(name="den", bufs=2))

                def dots_mms(pw, g, hs, ksl, qsl, sub):
                    nc.tensor.matmul(
                        pw[:, hs, g, sub],
                        kt2[64 * hs:64 * hs + 64, g, ksl],
                        qt2[64 * hs:64 * hs + 64, g, qsl],
                        start=True, stop=True)

                def stage_a(si):
                    """dots + exp + mask for batch si -> em tile;
                    also projects the two v chunks stage_b(si) will need
                    beyond what earlier stage_a calls produced."""
                    s = si * 256
                    v_pair(2 * si + 1)
                    em = epool.tile([128, 3, 4, 256], f16, tag="em",
                                    name=f"em{si}")
                    # chunk 0 = fused edges (c0 at p 0:128, c3 at p 128:256),
                    # chunks 1/2 = middles.  exp+mask are emitted right after
                    # each chunk's matmuls so the ring-2 pw pool can recycle.
                    # er12 collects both middle exps so their mask multiply
                    # runs as one 2x-mode DVE op; edge mask goes to GpSimd.
                    er12 = erpool.tile([128, 2, 4, 256], f16, tag="er12")
                    for idx in range(3):
                        pt = pwp.tile([128, 2, 2, 256], f32, tag="pw")
                        for g in range(2):
                            for hs in range(2):
                                if idx == 0:
                                    dots_mms(pt, g, hs,
                                             slice(s, s + 128),
                                             slice(s, s + 128),
                                             slice(0, 128))
                                    dots_mms(pt, g, hs,
                                             slice(s + 384, s + 512),
                                             slice(s + 128, s + 256),
                                             slice(128, 256))
                                else:
                                    dots_mms(pt, g, hs,
                                             slice(s + 128 * idx,
                                                   s + 128 * idx + 128),
                                             slice(s, s + 256),
                                             slice(0, 256))
                        if idx == 0:
                            er = erpool.tile([128, 4, 256], f16, tag="er")
                        else:
                            er = er12[:, idx - 1]
                        nc.scalar.activation(
                            out=er,
                            in_=pt.rearrange("p a b f -> p (a b) f"),
                            func=Exp, scale=SCALE)
                        if idx == 0:
                            nc.gpsimd.tensor_mul(em[:, 0], er, masksf[:, 0])
                    nc.vector.tensor_mul(em[:, 1:3], er12, masksf[:, 1:3])
                    return em

                def stage_b(si, em):
                    """AV + normalize + transpose + projection + out"""
                    s = si * 256
                    # ph stride padded to a full 2KB bank: a matmul PSUM
                    # target must not cross a bank boundary
                    po_raw = pso.tile([128, 2, 512], f32, tag="po")
                    po = po_raw[:, :, 0:260].rearrange(
                        "p a (g e) -> p a g e", e=65)
                    for gh in range(4):
                        slot = 2 * (gh % 2) + gh // 2
                        for ph in range(2):
                            # chunk index within em (0 = fused edge)
                            cs = ((0, 1, 2) if ph == 0 else (1, 2, 0))
                            vcs = ((0, 1, 2) if ph == 0 else (1, 2, 3))
                            for i in range(3):
                                nc.tensor.matmul(
                                    po[:, ph, gh, :],
                                    em[:, cs[i], slot,
                                       ph * 128:(ph + 1) * 128],
                                    vsb[:, 2 * si + vcs[i],
                                        65 * gh:65 * gh + 65],
                                    start=(i == 0), stop=(i == 2))
                    den = dpool.tile([128, 2, 4], f32, tag="den")
                    nc.vector.tensor_add(
                        den.unsqueeze(3),
                        po[:, :, :, 64:65],
                        npad.unsqueeze(2).unsqueeze(3).to_broadcast(
                            [128, 2, 4, 1]))
                    rec = dpool.tile([128, 2, 4], f32, tag="rec")
                    nc.vector.reciprocal(rec, den)

                    ob = opool.tile([128, 2, DIM], f16, tag="ob")
                    for ph in range(2):
                        opix = opool.tile([128, 256], f16, tag="opix")
                        nc.vector.tensor_mul(
                            opix.rearrange("p (g e) -> p g e", g=4),
                            po[:, ph, :, 0:64],
                            rec[:, ph, :].unsqueeze(2).to_broadcast(
                                [128, 4, 64]))
                        otb = opool.tile([128, 2, 128], f16, tag="otb")
                        # transposes land in po_raw's pad hole (disjoint from
                        # the attention columns) to save a PSUM bank
                        pt = po_raw[:, ph, 320:448].bitcast(f16).rearrange(
                            "p (i f) -> p i f", i=2)
                        for i in range(2):
                            nc.tensor.transpose(
                                pt[:, i], opix[:, i * 128:(i + 1) * 128],
                                ident)
                        nc.vector.tensor_copy(otb, pt)
                        pj = pspj.tile([128, DIM], f32, tag="pj")
                        for i in range(2):
                            nc.tensor.matmul(
                                pj, otb[:, i], woutt[:, i],
                                start=(i == 0), stop=(i == 1))
                        # casts alternate ACT/DVE for balance
                        if ph == 0:
                            nc.scalar.copy(ob[:, 0], pj)
                        else:
                            nc.vector.tensor_copy(ob[:, 1], pj)
                    # one DMA for both 128-px halves (fewer DMA instrs +
                    # semaphore waits)
                    nc.sync.dma_start(
                        out=out_d[s:s + 256, :].rearrange(
                            "(h p) m -> p h m", p=128),
                        in_=ob)

                # skewed software pipeline: dots/exp/mask of batch i+1 are
                # emitted before AV/proj of batch i so the PE never waits
                # on the ACT exp chain
                em_prev = stage_a(0)
                for si in range(16):
                    em_next = stage_a(si + 1) if si < 15 else None
                    stage_b(si, em_prev)
                    em_prev = em_next

    nc.finalize()
    return nc


def _prepare_core_inputs(x, w_qkv, w_out, b_out):
    masks3, n_pad = _make_masks()
    masks_p = np.ascontiguousarray(masks3)        # [128, 3, 256]
    ident = np.eye(128, dtype=np.float16)
    per_core = []
    for ci in range(N_CORES):
        b, hg = ci // 2, ci % 2
        q_rows = w_qkv[256 * hg:256 * hg + 256]
        k_rows = w_qkv[INNER + 256 * hg:INNER + 256 * hg + 256]
        v_rows = w_qkv[2 * INNER + 256 * hg:2 * INNER + 256 * hg + 256]
        w_slice = np.concatenate([q_rows, k_rows, v_rows], axis=0)  # [768, 512]
        xt = x[b].T.astype(np.float16)                       # [512, 4096]
        # blocked: [blk, p, c, n-slice] so each per-partition DMA run is 4KB
        xtb = np.ascontiguousarray(
            xt.reshape(4, 128, 8, 512).transpose(2, 1, 0, 3))
        wq = np.ascontiguousarray(
            w_slice.T.astype(np.float16).reshape(4, 128, 768).transpose(1, 0, 2))
        per_core.append({
            "xt": xtb,
            "wqkvt": wq,
            "woutt": np.ascontiguousarray(
                w_out[:, 256 * hg:256 * hg + 256].T).astype(np.float16),
            "masks": masks_p,
            "npad": n_pad,
            "ident": ident,
        })
    return per_core


def kernel(x, w_qkv, w_out, b_out, h, w):
    assert int(h) == HMAP and int(w) == WMAP
    x = np.asarray(x, dtype=np.float32)
    w_qkv = np.asarray(w_qkv, dtype=np.float32)
    w_out = np.asarray(w_out, dtype=np.float32)
    b_out = np.asarray(b_out, dtype=np.float32)

    if "nc" not in _cache:
        _cache["nc"] = _build_nc()
    nc = _cache["nc"]

    from concourse.bass_utils import run_bass_kernel_spmd
    in_maps = _prepare_core_inputs(x, w_qkv, w_out, b_out)
    res = run_bass_kernel_spmd(nc, in_maps, core_ids=list(range(N_CORES)))
    out = np.zeros((B, N, DIM), dtype=np.float32)
    for b in range(B):
        out[b] = (res.results[2 * b]["out"].astype(np.float32)
                  + res.results[2 * b + 1]["out"].astype(np.float32)
                  + b_out[None, :])
    return out
